# revision 9
# baseline (speedup 1.0000x reference)
# Trainium2 Bass kernel for nn_Net_4861902979707
#
# Computation (per sample, B = 4194304):
#   X [B, 3, 3] -> 3 pairwise Euclidean distances d = [d01, d02, d12]
#   h1 = elu(d @ W1.T + b1); h2 = elu(h1 @ W2.T + b2); y = h2 @ W3.T + b3
#
# Strategy: pure data parallel over 8 NeuronCores (batch split); fp16 I/O
# (X cast to fp16 on host, y returned fp16 and upcast) to halve HBM traffic
# in this memory-bound regime. Sample-major on-chip layout [128, T] tiles.
#
# Engine assignment (per tile, T=512):
#   DVE   : pairwise diffs + squares as fused scalar_tensor_tensor ops
#           (fp16/SBUF/packed -> 4x DVE mode), layer-2 FMA chains, elu2.
#   PE    : coord sums of squares (identity diag matmuls into PSUM fp32),
#           layer-1 as diag-matmul accumulation incl. bias via ones-column.
#   ACT   : ln(q), dist = exp(0.5 ln q) (sqrt via the natural_log_exp table
#           -- avoids the sqrt<->exp act-table thrash, 1.28us per reload),
#           exp for both ELU layers.
#   GPSIMD: elu1 combine, layer-3 FMA chain, output DMA queue.
#
# ELU identity used everywhere (exact, and exp overflow-safe):
#   elu(z)+1 = max(z+1, min(exp(z), 1))
# so each ELU is one ACT exp + one fused (min,max) scalar_tensor_tensor.
# The +1 shift is absorbed into the next layer's bias (b' = b - W @ 1), and
# z' = z + b + 1 is produced directly by the PE (bias+1 via ones column) or
# the DVE FMA chain; exp(z) = Exp(z' , bias=-1).
#
# The emission loop is software-pipelined (stages A / B1 / B2 with lag 1
# and 2) so every engine's in-order stream has its dependencies resolved
# ~a full tile ahead -- ACT streams back-to-back.
import os as _os
import numpy as np

B = 4194304
N_CORES = 8
B_CORE = B // N_CORES          # 524288
P = 128                        # partitions
T = int(_os.environ.get("TSZ", "512"))
TILE = P * T
N_TILES = B_CORE // TILE

COMPUTE_DT = "fp16"
XCAST = True                   # X delivered as fp16 (host cast)

H1_ENG = _os.environ.get("H1ENG", "dve")    # dve only (gpsimd cannot
                                            # access PSUM; z1' lives there)
L3_ENG = _os.environ.get("L3ENG", "dve")    # dve only (Pool engine cannot
                                            # run TensorScalarPtr)
BUFS_X = int(_os.environ.get("BUFS_X", "4"))
BUFS_W = int(_os.environ.get("BUFS_W", "3"))
BUFS_M = int(_os.environ.get("BUFS_M", "3"))

_CACHE = {}


def _split_sync_waits(nc, mybir, limit=1):
    """This walrus build rejects instructions carrying more than ~1 sem wait
    ("Too many sync wait commands"). Hoist excess waits onto NoOp carrier
    instructions (same engine, immediately before) -- engine program order
    preserves the blocking semantics."""
    n_split = 0
    for f in nc.m.functions:
        for b in f.blocks:
            lst = b.instructions
            out = []
            changed = False
            for inst in lst:
                si = inst.sync_info
                if si is not None and len(si.on_wait) > limit:
                    waits = list(si.on_wait)
                    extra, keep = waits[:-limit], waits[-limit:]
                    for wi, w in enumerate(extra):
                        nop = mybir.InstNoOp(
                            name=f"wsplit-{inst.name}-{wi}")
                        nop.engine = inst.engine
                        nop.sync_info = mybir.SyncInfo(
                            on_wait=[w], on_update=[])
                        out.append(nop)
                        n_split += 1
                    inst.sync_info = type(si)(
                        on_wait=keep, on_update=list(si.on_update))
                    changed = True
                out.append(inst)
            if changed:
                b.instructions = out
    return n_split


# WD diag-matrix indices (each a [128,128] fp16 lhsT)
def _iWD_I():
    return 0
def _iWD_W1(k, j):
    return 1 + 3 * k + j
def _iWD_B1(k):
    return 7 + k
N_WD = 9

# WB scalar indices: W2[m,j]=2m+j, b2'[m]+1=4+m, W3[j]=6+j, b3'=8, -1.0=9
def _iW2(m, j):
    return 2 * m + j
def _iB2(m):
    return 4 + m
def _iW3(j):
    return 6 + j
_IB3 = 8
_IM1 = 9
N_WB = 10


def _build(dt_name, reps=1, bench_small=False):
    import concourse.bass as bass
    import concourse.tile as tile
    import concourse.mybir as mybir

    f32 = mybir.dt.float32
    f16 = mybir.dt.float16
    Alu = mybir.AluOpType
    Act = mybir.ActivationFunctionType

    nc = bass.Bass()
    BC = TILE if bench_small else B_CORE
    X = nc.dram_tensor("X", [BC, 9], f16, kind="ExternalInput")
    WB = nc.dram_tensor("WB", [N_WB], f32, kind="ExternalInput")
    WD = nc.dram_tensor("WD", [N_WD, P, P], f16, kind="ExternalInput")
    Y = nc.dram_tensor("Y", [BC, 1], f16, kind="ExternalOutput")

    with tile.TileContext(nc) as tc:
        with (
            tc.tile_pool(name="singles", bufs=1) as singles,
            tc.tile_pool(name="xin", bufs=BUFS_X) as xin,
            tc.tile_pool(name="work", bufs=BUFS_W) as work,
            tc.tile_pool(name="mlp", bufs=BUFS_M) as mlp,
            tc.tile_pool(name="yout", bufs=3) as yout,
            tc.tile_pool(name="psq", bufs=2, space="PSUM") as psq,
            tc.tile_pool(name="psz", bufs=1, space="PSUM") as psz,
        ):
            # broadcast bias scalars to all partitions; load diag matrices
            wb = singles.tile([P, N_WB], f32)
            nc.gpsimd.dma_start(
                out=wb[:],
                in_=bass.AP(tensor=WB[:].tensor, offset=0,
                            ap=[[0, P], [1, N_WB]]))
            wd = singles.tile([P, N_WD, P], f16)
            nc.sync.dma_start(
                out=wd[:],
                in_=bass.AP(tensor=WD[:].tensor, offset=0,
                            ap=[[P, P], [P * P, N_WD], [1, P]]))
            ones = singles.tile([P, T], f16)
            nc.vector.memset(ones[:], 1.0)

            def ws(i):  # [P,1] fp32 bias/weight scalar AP
                return wb[:, i:i + 1]

            def diag(i):  # [128,128] fp16 lhsT AP
                return wd[:, i, :]

            eng_h1 = nc.gpsimd if H1_ENG == "pool" else nc.vector
            eng_l3 = nc.gpsimd if L3_ENG == "pool" else nc.vector

            # per-round live state (stage A results consumed by B1/B2)
            st = {}

            def stage_a(ti):
                src = 0 if bench_small else ti
                xr = X[src * TILE:(src + 1) * TILE, :].rearrange(
                    "(p s) d -> p s d", p=P)
                xt = xin.tile([P, T, 9], f16)
                nc.sync.dma_start(out=xt[:], in_=xr)

                # pairwise diffs, then squares in place; all ops are
                # fp16/SBUF/innermost-packed scalar_tensor_tensor (4x DVE)
                d = work.tile([P, T, 9], f16, tag="diff")
                nc.vector.scalar_tensor_tensor(
                    out=d[:, :, 0:3], in0=xt[:, :, 0:3], scalar=1.0,
                    in1=xt[:, :, 3:6], op0=Alu.mult, op1=Alu.subtract)
                nc.vector.scalar_tensor_tensor(
                    out=d[:, :, 3:6], in0=xt[:, :, 0:3], scalar=1.0,
                    in1=xt[:, :, 6:9], op0=Alu.mult, op1=Alu.subtract)
                nc.vector.scalar_tensor_tensor(
                    out=d[:, :, 6:9], in0=d[:, :, 3:6], scalar=1.0,
                    in1=d[:, :, 0:3], op0=Alu.mult, op1=Alu.subtract)
                nc.vector.scalar_tensor_tensor(
                    out=d[:], in0=d[:], scalar=1.0, in1=d[:],
                    op0=Alu.mult, op1=Alu.mult)

                # coord sums of squares on PE -> q [P, 3, T] PSUM fp32
                q = psq.tile([P, 3, T], f32, tag="q")
                for pi in range(3):
                    for c in range(3):
                        nc.tensor.matmul(
                            q[:, pi, :], diag(_iWD_I()), d[:, :, 3 * pi + c],
                            start=(c == 0), stop=(c == 2))

                # dist = exp(0.5 * ln q): both from the natural_log_exp
                # act table -- no table reloads anywhere in the kernel
                lnq = mlp.tile([P, 3, T], f16, tag="lnq")
                nc.scalar.activation(lnq[:], q[:], Act.Ln)
                dist = mlp.tile([P, 3, T], f16, tag="dist")
                nc.scalar.activation(dist[:], lnq[:], Act.Exp, scale=0.5)
                st[ti] = {"dist": dist}

            def stage_b1(ti):
                s = st[ti]
                dist = s["dist"]
                # L1 on PE: z1'[k] = sum_j W1[k,j] dist_j + (b1[k]+1)
                z1 = psz.tile([P, 2, T], f32, tag="z1")
                for k in range(2):
                    for j in range(3):
                        nc.tensor.matmul(
                            z1[:, k, :], diag(_iWD_W1(k, j)), dist[:, j, :],
                            start=(j == 0), stop=False)
                    nc.tensor.matmul(
                        z1[:, k, :], diag(_iWD_B1(k)), ones[:],
                        start=False, stop=True)
                e1 = mlp.tile([P, 2, T], f16, tag="e1")
                nc.scalar.activation(e1[:], z1[:], Act.Exp, bias=ws(_IM1))
                s["z1"] = z1
                s["e1"] = e1

            def stage_b2(ti):
                s = st.pop(ti)
                z1, e1 = s["z1"], s["e1"]
                # elu1+1 = max(z1', min(e1, 1))
                h1 = mlp.tile([P, 2, T], f16, tag="h1")
                eng_h1.scalar_tensor_tensor(
                    out=h1[:], in0=e1[:], scalar=1.0, in1=z1[:],
                    op0=Alu.min, op1=Alu.max)
                # L2 on DVE: z2'[m] = sum_j W2[m,j] h1_j + (b2'[m]+1)
                z2 = mlp.tile([P, 2, T], f16, tag="z2")
                for m in range(2):
                    nc.vector.tensor_scalar(
                        out=z2[:, m, :], in0=h1[:, 0, :],
                        scalar1=ws(_iW2(m, 0)), scalar2=ws(_iB2(m)),
                        op0=Alu.mult, op1=Alu.add)
                    nc.vector.scalar_tensor_tensor(
                        out=z2[:, m, :], in0=h1[:, 1, :],
                        scalar=ws(_iW2(m, 1)), in1=z2[:, m, :],
                        op0=Alu.mult, op1=Alu.add)
                e2 = mlp.tile([P, 2, T], f16, tag="e2")
                nc.scalar.activation(e2[:], z2[:], Act.Exp, bias=ws(_IM1))
                h2 = mlp.tile([P, 2, T], f16, tag="h2")
                nc.vector.scalar_tensor_tensor(
                    out=h2[:], in0=e2[:], scalar=1.0, in1=z2[:],
                    op0=Alu.min, op1=Alu.max)
                # L3: y = W3[0] h2_0 + W3[1] h2_1 + b3'
                yt = yout.tile([P, T], f16)
                eng_l3.tensor_scalar(
                    out=yt[:], in0=h2[:, 0, :], scalar1=ws(_iW3(0)),
                    scalar2=ws(_IB3), op0=Alu.mult, op1=Alu.add)
                eng_l3.scalar_tensor_tensor(
                    out=yt[:], in0=h2[:, 1, :], scalar=ws(_iW3(1)),
                    in1=yt[:], op0=Alu.mult, op1=Alu.add)
                src = 0 if bench_small else ti
                yr = Y[src * TILE:(src + 1) * TILE, :].rearrange(
                    "(p s) d -> p (s d)", p=P)
                nc.sync.dma_start(out=yr, in_=yt[:])

            # reps>1 wraps the whole body in a For_i loop (benchmarking only)
            _loop = tc.For_i(0, reps) if reps != 1 else None
            if _loop is not None:
                _loop.__enter__()

            for r in range(N_TILES + 2):
                if r < N_TILES:
                    stage_a(r)
                if 0 <= r - 1 < N_TILES:
                    stage_b1(r - 1)
                if 0 <= r - 2 < N_TILES:
                    stage_b2(r - 2)

            if _loop is not None:
                _loop.__exit__(None, None, None)

    _split_sync_waits(nc, mybir, limit=1)
    return nc


def _pack_weights(W1, b1, W2, b2, W3, b3):
    W1 = np.asarray(W1, np.float32); b1 = np.asarray(b1, np.float32)
    W2 = np.asarray(W2, np.float32); b2 = np.asarray(b2, np.float32)
    W3 = np.asarray(W3, np.float32); b3 = np.asarray(b3, np.float32)
    b2a = b2 - W2.sum(axis=1)            # absorb elu(+1) shift
    b3a = b3 - W3.sum(axis=1)

    wb = np.empty(N_WB, np.float32)
    for m in range(2):
        for j in range(2):
            wb[_iW2(m, j)] = W2[m, j]
        wb[_iB2(m)] = b2a[m] + 1.0       # +1 for the max(z+1, .) elu form
    for j in range(2):
        wb[_iW3(j)] = W3[0, j]
    wb[_IB3] = b3a[0]
    wb[_IM1] = -1.0

    eye = np.eye(P, dtype=np.float32)
    wdf = np.empty((N_WD, P, P), np.float32)
    wdf[_iWD_I()] = eye
    for k in range(2):
        for j in range(3):
            wdf[_iWD_W1(k, j)] = eye * W1[k, j]
        wdf[_iWD_B1(k)] = eye * (b1[k] + 1.0)
    return wb, wdf.astype(np.float16)


LAST_RESULTS = None  # BassKernelResults of the most recent run (for test.py)


def kernel(X, W1, b1, W2, b2, W3, b3):
    from concourse.bass_utils import run_bass_kernel_spmd
    global LAST_RESULTS

    X = np.ascontiguousarray(
        np.asarray(X, np.float32).reshape(B, 9)).astype(np.float16)
    wb, wd = _pack_weights(W1, b1, W2, b2, W3, b3)

    key = (COMPUTE_DT, 1)
    if key not in _CACHE:
        _CACHE[key] = _build(COMPUTE_DT)
    nc = _CACHE[key]

    in_maps = [
        {"X": X[c * B_CORE:(c + 1) * B_CORE], "WB": wb, "WD": wd}
        for c in range(N_CORES)
    ]
    res = run_bass_kernel_spmd(nc, in_maps, core_ids=list(range(N_CORES)))
    LAST_RESULTS = res
    out = np.concatenate([res.results[c]["Y"] for c in range(N_CORES)], axis=0)
    return out.reshape(B, 1).astype(np.float32)
